# revision 26
# baseline (speedup 1.0000x reference)
"""ContMix kernel for TRN2, 8 NeuronCores.

Sharding: (batch b, H-half) -> 8 cores. Each core computes out[b, :, r0:r0+28, :].

Math (per batch b):
  ctx_p = avgpool8x8(ctx)                      [C, 49]   (DVE f16 tree-adds, 4x mode)
  G     = (Wq^T Wk / 64) @ ctx_p               [C, 49]   (weights folded on host)
  lg    = G^T @ x                              [49, HW]  (5 pair-aligned psum chunks)
  A     = softmax_s(lg)  (no max-sub; |lg| ~ 8), expa f16
  dynT  = A^T @ Wwd^T                          [112, 26] per row-pair
  out[c, n] = sum_j x_patch[c, j, n] * dyn[j, n]
Final step: banded matmuls on TensorE. Per row-pair a [120 x 112] banded
matrix M is built fully on-chip: gpsimd local_scatter writes M^T for TWO
pairs per call, then TensorE transpose-mode matmuls (identity rhs)
produce M chunks in PSUM (f16), copied to SBUF by DVE. Output copies on
ACT, streamed f16 output DMA, host upcasts to f32.
"""

import numpy as np

B, C, H, W = 4, 384, 56, 56
KK, S = 5, 7
NCORES = 8
ROWS = H // 2              # 28 rows per core
NPIX = ROWS * W            # 1568
PADR = ROWS + 4            # 32 padded rows
PADW = 60                  # padded width (56 + 4 halo)
WSPACE = PADR * PADW       # 1920 padded pixels
PCHUNK = 2 * PADW          # 120 partitions per contraction chunk
NPAIR = ROWS // 2          # 14 output row-pairs
NCHUNK = PADR // 2         # 16 contraction chunks
D2 = C // 2                # 192
NI = 26                    # scatter idxs per pair (25 taps + 1 pad)
NI2 = 2 * NI               # two pairs per scatter call
MCOLS = 3 * PCHUNK         # 360 = w''-space per pair
HW = H * W

_cached = {}


def _build_nc():
    import concourse.tile as tile
    from concourse import bacc, mybir, library_config

    f32, f16, i16 = mybir.dt.float32, mybir.dt.float16, mybir.dt.int16
    nc = bacc.Bacc("TRN2", target_bir_lowering=False, debug=False)

    ct_d = nc.dram_tensor("ct", [128, 25 * C], f16, kind="ExternalInput")
    bt_d = nc.dram_tensor("bt", [128, 25 * S * S], f16, kind="ExternalInput")
    wkq_d = nc.dram_tensor("wkq", [128, 3 * C], f16, kind="ExternalInput")
    wwdt1_d = nc.dram_tensor("wwdt1", [S * S, NI], f16, kind="ExternalInput")
    sidx2_d = nc.dram_tensor("sidx2", [2 * W, NI2], i16, kind="ExternalInput")
    ident_d = nc.dram_tensor("ident", [2 * W, 2 * W], f16, kind="ExternalInput")
    xn_d = nc.dram_tensor("xn", [C, NPIX], f16, kind="ExternalInput")
    xt_d = nc.dram_tensor("xt", [WSPACE, C], f16, kind="ExternalInput")
    out_d = nc.dram_tensor("out", [C, NPIX], f16, kind="ExternalOutput")

    # logits chunks (pair-aligned; small first to launch the scatter chain asap)
    LCH = [(0, 224), (224, 224), (448, 448), (896, 448), (1344, 224)]
    # output DMA chunks
    OCH = [(0, 784), (784, 560), (1344, 112), (1456, 112)]

    with tile.TileContext(nc) as tc:
        with (
            tc.tile_pool(name="big", bufs=1) as big,
            tc.tile_pool(name="wrk", bufs=4) as wrk,
            tc.tile_pool(name="mtp", bufs=3) as mtp,
            tc.tile_pool(name="psA", bufs=2, space="PSUM") as psA,
            tc.tile_pool(name="psD", bufs=1, space="PSUM") as psD,
            tc.tile_pool(name="psT", bufs=2, space="PSUM") as psT,
            tc.tile_pool(name="psO", bufs=3, space="PSUM") as psO,
        ):
            # ---------------- input DMAs (pooling chain first) ----------------
            bt_sb = big.tile([128, 25, S * S], f16, tag="bt")
            ident_sb = big.tile([2 * W, 2 * W], f16, tag="ident")
            ct_sb = big.tile([128, 25, C], f16, tag="ct")

            def ct_dma(k0, nk):
                nc.sync.dma_start(out=ct_sb[:, k0:k0 + nk, :],
                                  in_=ct_d[:, k0 * C:(k0 + nk) * C].rearrange(
                                      "p (k c) -> p k c", k=nk))

            ct_dma(0, 7)
            nc.sync.dma_start(out=bt_sb[:], in_=bt_d[:].rearrange("p (k s) -> p k s", k=25))
            nc.sync.dma_start(out=ident_sb[:], in_=ident_d[:])
            ct_dma(7, 6)
            ct_dma(13, 6)
            ct_dma(19, 6)
            wkq_sb = big.tile([128, 3, C], f16, tag="wkq")
            nc.sync.dma_start(out=wkq_sb[:], in_=wkq_d[:].rearrange("p (u c) -> p u c", u=3))
            wwdt1_sb = big.tile([S * S, NI], f16, tag="wwdt1")
            nc.sync.dma_start(out=wwdt1_sb[:], in_=wwdt1_d[:])
            sidx2_sb = big.tile([2 * W, NI2], i16, tag="sidx2")
            nc.sync.dma_start(out=sidx2_sb[:], in_=sidx2_d[:])
            xn_sb = big.tile([128, 3, NPIX], f16, tag="xn")
            for off, wdt in ((0, 448), (448, 448), (896, 448), (1344, 224)):
                nc.sync.dma_start(out=xn_sb[:, :, off:off + wdt],
                                  in_=xn_d[:, off:off + wdt].rearrange("(u p) n -> p u n", p=128))
            xt_sb = big.tile([PCHUNK, NCHUNK, C], f16, tag="xt")
            nc.sync.dma_start(out=xt_sb[:, 0:8, :],
                              in_=xt_d[0:8 * PCHUNK, :].rearrange("(t p) c -> p t c", p=PCHUNK))
            nc.sync.dma_start(out=xt_sb[:, 8:16, :],
                              in_=xt_d[8 * PCHUNK:, :].rearrange("(t p) c -> p t c", p=PCHUNK))

            nc.gpsimd.load_library(library_config.local_scatter)

            # -------- pooling on TensorE: ctx_p^T = B^T @ ctx^T ---------------
            # (contraction over hw positions; one-hot bin matrix B, 25 chunks)
            pool_ps = psA.tile([S * S, C], f32, tag="psA", name="pool_ps")
            for k in range(25):
                nc.tensor.matmul(pool_ps[:], bt_sb[:, k, :], ct_sb[:, k, :],
                                 start=(k == 0), stop=(k == 24))
            ctxpT = big.tile([S * S, C], f16, tag="ctxpT")
            nc.vector.tensor_copy(ctxpT[:, 0:192], pool_ps[:, 0:192])
            nc.scalar.copy(ctxpT[:, 192:C], pool_ps[:, 192:C])
            # transpose [49, 384] -> [384, 49] via 3 PE transpose matmuls
            ctp_ps = psT.tile([128, 3, 50], f16, tag="psT", name="ctp_ps")
            for cb in range(3):
                nc.tensor.matmul(ctp_ps[:, cb, 0:S * S], ctxpT[:, cb * 128:(cb + 1) * 128],
                                 ident_sb[0:S * S, 0:S * S], start=True, stop=True,
                                 is_transpose=True)
            ctx_p = big.tile([128, 3, S * S], f16, tag="ctxp")
            nc.vector.tensor_copy(ctx_p[:], ctp_ps[:, :, 0:S * S])

            # ------------- G = (Wq^T Wk/64) @ ctx_p : [384, 49], f16 ----------
            g_sb = big.tile([128, 3, S * S], f16, tag="g")
            g_ps = psA.tile([128, 3, S * S], f32, tag="psA", name="g_ps")
            for cb in range(3):
                for u in range(3):
                    nc.tensor.matmul(g_ps[:, cb, :], wkq_sb[:, u, cb * 128:(cb + 1) * 128],
                                     ctx_p[:, u, :], start=(u == 0), stop=(u == 2))
            nc.scalar.copy(g_sb[:], g_ps[:])

            # ------------- logits + exp (f16), dyn/d16 interleaved ------------
            expa = big.tile([S * S, NPIX], f16, tag="expa")
            dyn_ps = [None] * NPAIR
            d2_sb = [None] * (NPAIR // 2)

            def emit_lg(k):
                off, wdt = LCH[k]
                lg = psA.tile([S * S, 448], f32, tag="psA", name=f"lg{k}")
                for u in range(3):
                    nc.tensor.matmul(lg[:, 0:wdt], g_sb[:, u, :], xn_sb[:, u, off:off + wdt],
                                     start=(u == 0), stop=(u == 2))
                nc.scalar.activation(expa[:, off:off + wdt], lg[:, 0:wdt],
                                     mybir.ActivationFunctionType.Exp)

            def emit_dyn(p):
                dp = psD.tile([2 * W, NI], f32, tag="psD", name=f"dyn{p}")
                nc.tensor.matmul(dp[:], expa[:, p * 112:(p + 1) * 112],
                                 wwdt1_sb[:], start=True, stop=True)
                dyn_ps[p] = dp

            def emit_d16(p):
                g2 = p // 2
                if d2_sb[g2] is None:
                    d2_sb[g2] = wrk.tile([2 * W, NI2], f16, tag="d2", name=f"d2_{g2}")
                rec = wrk.tile([2 * W, 1], f32, tag="rec", name=f"rec{p}")
                nc.vector.reciprocal(rec[:], dyn_ps[p][:, 25:26])
                nc.vector.tensor_scalar_mul(
                    d2_sb[g2][:, (p % 2) * NI:(p % 2 + 1) * NI], dyn_ps[p][:], rec[:])

            mt2_sb = [None] * (NPAIR // 2)

            def emit_scatter(g2):
                mt2 = mtp.tile([2 * W, 2 * MCOLS], f16, tag="mt2", name=f"mt2_{g2}")
                nc.gpsimd.local_scatter(mt2[:], d2_sb[g2][:], sidx2_sb[:],
                                        channels=2 * W, num_elems=2 * MCOLS, num_idxs=NI2)
                mt2_sb[g2] = mt2

            emit_lg(0)
            emit_lg(1)
            for p in (0, 1):
                emit_dyn(p)
            emit_lg(2)
            for p in (0, 1):
                emit_d16(p)
            emit_scatter(0)
            for p in (2, 3):
                emit_dyn(p)
            emit_lg(3)
            for p in (2, 3):
                emit_d16(p)
            emit_scatter(1)
            for p in (4, 5, 6, 7):
                emit_dyn(p)
            emit_lg(4)
            for p in (4, 5):
                emit_d16(p)
            emit_scatter(2)
            for p in (6, 7):
                emit_d16(p)
            emit_scatter(3)
            for p in (8, 9, 10, 11, 12, 13):
                emit_dyn(p)
            for p in (8, 9):
                emit_d16(p)
            emit_scatter(4)
            for p in (10, 11):
                emit_d16(p)
            emit_scatter(5)
            for p in (12, 13):
                emit_d16(p)
            emit_scatter(6)

            # --------- PE: transpose M chunks + banded matmuls, pipelined ------
            m_sb = big.tile([PCHUNK, NPAIR, 3, 112], f16, tag="m")
            out_sb = big.tile([128, 3, NPIX], f16, tag="out")

            def emit_T(p):
                tp = psT.tile([PCHUNK, 3, 112], f16, tag="psT", name=f"tp{p}")
                src = mt2_sb[p // 2]
                base = (p % 2) * MCOLS
                for t3 in range(3):
                    nc.tensor.matmul(tp[:, t3, :],
                                     src[:, base + t3 * PCHUNK:base + (t3 + 1) * PCHUNK],
                                     ident_sb[:], start=True, stop=True,
                                     is_transpose=True)
                nc.vector.tensor_copy(m_sb[:, p, :, :], tp[:])

            def emit_B(p):
                po = psO.tile([128, 3, 112], f32, tag="psO", name=f"po{p}")
                for cc in range(3):
                    for trel in range(3):
                        nc.tensor.matmul(po[:, cc, :],
                                         xt_sb[:, p + trel, cc * 128:(cc + 1) * 128],
                                         m_sb[:, p, trel, :],
                                         start=(trel == 0), stop=(trel == 2))
                dst = out_sb[:, :, p * 112:(p + 1) * 112]
                if p % 2 == 0:
                    nc.scalar.copy(dst, po[:])
                else:
                    nc.vector.tensor_copy(dst, po[:])

            emit_T(0)
            emit_T(1)
            for p in range(2, NPAIR):
                emit_T(p)
                emit_B(p - 2)
            emit_B(NPAIR - 2)
            emit_B(NPAIR - 1)

            # ---------------- streamed output DMAs ----------------
            for off, wdt in OCH:
                nc.sync.dma_start(
                    out=out_d[:, off:off + wdt].rearrange("(u p) n -> p u n", p=128),
                    in_=out_sb[:, :, off:off + wdt])
    nc.finalize()
    return nc


def _pool_matrix():
    bt = np.zeros((3200, S * S), np.float16)
    r = np.arange(HW)
    bt[r, (r // 56 // 8) * 7 + (r % 56) // 8] = 1.0
    return np.ascontiguousarray(bt.reshape(25, 128, S * S).transpose(1, 0, 2)).reshape(128, -1)


def _static_inputs():
    sidx = np.full((2 * W, NI), -1, np.int16)
    for hl in range(2):
        for w in range(W):
            for di in range(KK):
                for dj in range(KK):
                    sidx[hl * W + w, 5 * di + dj] = (hl + di) * PADW + w + dj
    sidx2 = np.full((2 * W, NI2), -1, np.int16)
    sidx2[:, 0:NI] = sidx
    sidx2[:, NI:NI2] = np.where(sidx >= 0, sidx + MCOLS, -1).astype(np.int16)
    return sidx2


def _prep(x, ctx, Wq, Wk, Wwd):
    sidx2 = _static_inputs()
    # wkq[c', c] = (Wk^T @ Wq / 64); lhsT chunks packed [128, u, c]
    wkq = (Wk.T.astype(np.float64) @ Wq.astype(np.float64) / 64.0).astype(np.float16)
    wkq = np.ascontiguousarray(wkq.reshape(3, 128, C).transpose(1, 0, 2)).reshape(128, 3 * C)
    wwdt1 = np.concatenate([Wwd.T, np.ones((S * S, 1), np.float32)], axis=1).astype(np.float16)
    ident = np.eye(2 * W, dtype=np.float16)
    bt = _pool_matrix()
    in_maps = []
    for core in range(NCORES):
        b, half = core // 2, core % 2
        r0 = half * ROWS
        ct = np.zeros((3200, C), np.float16)
        ct[0:HW] = ctx[b].reshape(C, HW).T
        ct = np.ascontiguousarray(ct.reshape(25, 128, C).transpose(1, 0, 2)).reshape(128, -1)
        xn = np.ascontiguousarray(x[b, :, r0:r0 + ROWS, :].reshape(C, NPIX)).astype(np.float16)
        xp = np.zeros((PADR, PADW, C), np.float32)
        lo, hi = max(0, r0 - 2), min(H, r0 + ROWS + 2)
        xp[lo - (r0 - 2):hi - (r0 - 2), 2:2 + W, :] = np.transpose(x[b, :, lo:hi, :], (1, 2, 0))
        xt = xp.reshape(WSPACE, C).astype(np.float16)
        in_maps.append(dict(ct=ct, bt=bt, wkq=wkq, wwdt1=wwdt1, sidx2=sidx2,
                            ident=ident, xn=xn, xt=xt))
    return in_maps


def kernel(x, ctx, Wq, Wk, Wwd, _trace=False):
    from concourse.bass_utils import run_bass_kernel_spmd

    x, ctx = np.asarray(x), np.asarray(ctx)
    Wq, Wk, Wwd = np.asarray(Wq), np.asarray(Wk), np.asarray(Wwd)
    if "nc" not in _cached:
        _cached["nc"] = _build_nc()
    in_maps = _prep(x, ctx, Wq, Wk, Wwd)
    res = run_bass_kernel_spmd(_cached["nc"], in_maps, list(range(NCORES)), trace=_trace)
    _cached["last_result"] = res
    out = np.empty((B, C, H, W), np.float32)
    for core in range(NCORES):
        b, half = core // 2, core % 2
        r0 = half * ROWS
        out[b, :, r0:r0 + ROWS, :] = res.results[core]["out"].astype(np.float32).reshape(C, ROWS, W)
    return out


# revision 27
# speedup vs baseline: 1.0934x; 1.0934x over previous
"""ContMix kernel for TRN2, 8 NeuronCores.

Sharding: (batch b, H-half) -> 8 cores. Each core computes out[b, :, r0:r0+28, :].

Math (per batch b):
  ctx_p = avgpool8x8(ctx)                      [C, 49]   (DVE f16 tree-adds, 4x mode)
  G     = (Wq^T Wk / 64) @ ctx_p               [C, 49]   (weights folded on host)
  lg    = G^T @ x                              [49, HW]  (5 pair-aligned psum chunks)
  A     = softmax_s(lg)  (no max-sub; |lg| ~ 8), expa f16
  dynT  = A^T @ Wwd^T                          [112, 26] per row-pair
  out[c, n] = sum_j x_patch[c, j, n] * dyn[j, n]
Final step: banded matmuls on TensorE. Per row-pair a [120 x 112] banded
matrix M is built fully on-chip: gpsimd local_scatter writes M^T for TWO
pairs per call, then TensorE transpose-mode matmuls (identity rhs)
produce M chunks in PSUM (f16), copied to SBUF by DVE. Output copies on
ACT, streamed f16 output DMA, host upcasts to f32.
"""

import numpy as np

B, C, H, W = 4, 384, 56, 56
KK, S = 5, 7
NCORES = 8
ROWS = H // 2              # 28 rows per core
NPIX = ROWS * W            # 1568
PADR = ROWS + 4            # 32 padded rows
PADW = 60                  # padded width (56 + 4 halo)
WSPACE = PADR * PADW       # 1920 padded pixels
PCHUNK = 2 * PADW          # 120 partitions per contraction chunk
NPAIR = ROWS // 2          # 14 output row-pairs
NCHUNK = PADR // 2         # 16 contraction chunks
D2 = C // 2                # 192
NI = 26                    # scatter idxs per pair (25 taps + 1 pad)
NI2 = 2 * NI               # two pairs per scatter call
MCOLS = 3 * PCHUNK         # 360 = w''-space per pair
HW = H * W

_cached = {}


def _build_nc():
    import concourse.tile as tile
    from concourse import bacc, mybir, library_config

    f32, f16, i16 = mybir.dt.float32, mybir.dt.float16, mybir.dt.int16
    nc = bacc.Bacc("TRN2", target_bir_lowering=False, debug=False)

    ct_d = nc.dram_tensor("ct", [128, 25 * C], f16, kind="ExternalInput")
    bt_d = nc.dram_tensor("bt", [128, 25 * S * S], f16, kind="ExternalInput")
    wkq_d = nc.dram_tensor("wkq", [128, 3 * C], f16, kind="ExternalInput")
    wwdt1_d = nc.dram_tensor("wwdt1", [S * S, NI], f16, kind="ExternalInput")
    sidx2_d = nc.dram_tensor("sidx2", [2 * W, NI2], i16, kind="ExternalInput")
    ident_d = nc.dram_tensor("ident", [2 * W, 2 * W], f16, kind="ExternalInput")
    xn_d = nc.dram_tensor("xn", [C, NPIX], f16, kind="ExternalInput")
    xt_d = nc.dram_tensor("xt", [WSPACE, C], f16, kind="ExternalInput")
    out_d = nc.dram_tensor("out", [C, NPIX], f16, kind="ExternalOutput")

    # logits chunks (pair-aligned; small first to launch the scatter chain asap)
    LCH = [(0, 224), (224, 224), (448, 448), (896, 448), (1344, 224)]
    # output DMA chunks
    OCH = [(0, 784), (784, 560), (1344, 112), (1456, 112)]

    with tile.TileContext(nc) as tc:
        with (
            tc.tile_pool(name="big", bufs=1) as big,
            tc.tile_pool(name="wrk", bufs=4) as wrk,
            tc.tile_pool(name="mtp", bufs=3) as mtp,
            tc.tile_pool(name="psA", bufs=2, space="PSUM") as psA,
            tc.tile_pool(name="psD", bufs=2, space="PSUM") as psD,
            tc.tile_pool(name="psT", bufs=2, space="PSUM") as psT,
            tc.tile_pool(name="psO", bufs=2, space="PSUM") as psO,
        ):
            # ---------------- input DMAs (pooling chain first) ----------------
            bt_sb = big.tile([128, 25, S * S], f16, tag="bt")
            ident_sb = big.tile([2 * W, 2 * W], f16, tag="ident")
            ct_sb = big.tile([128, 25, C], f16, tag="ct")

            def ct_dma(k0, nk):
                nc.sync.dma_start(out=ct_sb[:, k0:k0 + nk, :],
                                  in_=ct_d[:, k0 * C:(k0 + nk) * C].rearrange(
                                      "p (k c) -> p k c", k=nk))

            ct_dma(0, 7)
            nc.sync.dma_start(out=bt_sb[:], in_=bt_d[:].rearrange("p (k s) -> p k s", k=25))
            nc.sync.dma_start(out=ident_sb[:], in_=ident_d[:])
            ct_dma(7, 6)
            ct_dma(13, 6)
            ct_dma(19, 6)
            wkq_sb = big.tile([128, 3, C], f16, tag="wkq")
            nc.sync.dma_start(out=wkq_sb[:], in_=wkq_d[:].rearrange("p (u c) -> p u c", u=3))
            wwdt1_sb = big.tile([S * S, NI], f16, tag="wwdt1")
            nc.sync.dma_start(out=wwdt1_sb[:], in_=wwdt1_d[:])
            sidx2_sb = big.tile([2 * W, NI2], i16, tag="sidx2")
            nc.sync.dma_start(out=sidx2_sb[:], in_=sidx2_d[:])
            xn_sb = big.tile([128, 3, NPIX], f16, tag="xn")
            for off, wdt in ((0, 448), (448, 448), (896, 448), (1344, 224)):
                nc.sync.dma_start(out=xn_sb[:, :, off:off + wdt],
                                  in_=xn_d[:, off:off + wdt].rearrange("(u p) n -> p u n", p=128))
            xt_sb = big.tile([PCHUNK, NCHUNK, C], f16, tag="xt")
            nc.sync.dma_start(out=xt_sb[:, 0:8, :],
                              in_=xt_d[0:8 * PCHUNK, :].rearrange("(t p) c -> p t c", p=PCHUNK))
            nc.sync.dma_start(out=xt_sb[:, 8:16, :],
                              in_=xt_d[8 * PCHUNK:, :].rearrange("(t p) c -> p t c", p=PCHUNK))

            nc.gpsimd.load_library(library_config.local_scatter)

            # -------- pooling on TensorE: ctx_p^T = B^T @ ctx^T ---------------
            # (contraction over hw positions; one-hot bin matrix B, 25 chunks)
            pool_ps = psA.tile([S * S, C], f32, tag="psA", name="pool_ps")
            for k in range(25):
                nc.tensor.matmul(pool_ps[:], bt_sb[:, k, :], ct_sb[:, k, :],
                                 start=(k == 0), stop=(k == 24))
            ctxpT = big.tile([S * S, C], f16, tag="ctxpT")
            nc.vector.tensor_copy(ctxpT[:, 0:192], pool_ps[:, 0:192])
            nc.scalar.copy(ctxpT[:, 192:C], pool_ps[:, 192:C])
            # transpose [49, 384] -> [384, 49] via 3 PE transpose matmuls
            ctp_ps = psT.tile([128, 3, 50], f16, tag="psT", name="ctp_ps")
            for cb in range(3):
                nc.tensor.matmul(ctp_ps[:, cb, 0:S * S], ctxpT[:, cb * 128:(cb + 1) * 128],
                                 ident_sb[0:S * S, 0:S * S], start=True, stop=True,
                                 is_transpose=True)
            ctx_p = big.tile([128, 3, S * S], f16, tag="ctxp")
            nc.vector.tensor_copy(ctx_p[:], ctp_ps[:, :, 0:S * S])

            # ------------- G = (Wq^T Wk/64) @ ctx_p : [384, 49], f16 ----------
            g_sb = big.tile([128, 3, S * S], f16, tag="g")
            g_ps = psA.tile([128, 3, S * S], f32, tag="psA", name="g_ps")
            for cb in range(3):
                for u in range(3):
                    nc.tensor.matmul(g_ps[:, cb, :], wkq_sb[:, u, cb * 128:(cb + 1) * 128],
                                     ctx_p[:, u, :], start=(u == 0), stop=(u == 2))
            nc.scalar.copy(g_sb[:], g_ps[:])

            # ------------- logits + exp (f16), dyn/d16 interleaved ------------
            expa = big.tile([S * S, NPIX], f16, tag="expa")
            dyn_ps = [None] * NPAIR
            d2_sb = [None] * (NPAIR // 2)

            def emit_lg(k):
                off, wdt = LCH[k]
                lg = psA.tile([S * S, 448], f32, tag="psA", name=f"lg{k}")
                for u in range(3):
                    nc.tensor.matmul(lg[:, 0:wdt], g_sb[:, u, :], xn_sb[:, u, off:off + wdt],
                                     start=(u == 0), stop=(u == 2))
                nc.scalar.activation(expa[:, off:off + wdt], lg[:, 0:wdt],
                                     mybir.ActivationFunctionType.Exp)

            def emit_dyn(p):
                dp = psD.tile([2 * W, NI], f32, tag="psD", name=f"dyn{p}")
                nc.tensor.matmul(dp[:], expa[:, p * 112:(p + 1) * 112],
                                 wwdt1_sb[:], start=True, stop=True)
                dyn_ps[p] = dp

            def emit_d16(p):
                g2 = p // 2
                if d2_sb[g2] is None:
                    d2_sb[g2] = wrk.tile([2 * W, NI2], f16, tag="d2", name=f"d2_{g2}")
                rec = wrk.tile([2 * W, 1], f32, tag="rec", name=f"rec{p}")
                nc.vector.reciprocal(rec[:], dyn_ps[p][:, 25:26])
                nc.vector.tensor_scalar_mul(
                    d2_sb[g2][:, (p % 2) * NI:(p % 2 + 1) * NI], dyn_ps[p][:], rec[:])

            mt2_sb = [None] * (NPAIR // 2)

            def emit_scatter(g2):
                mt2 = mtp.tile([2 * W, 2 * MCOLS], f16, tag="mt2", name=f"mt2_{g2}")
                nc.gpsimd.local_scatter(mt2[:], d2_sb[g2][:], sidx2_sb[:],
                                        channels=2 * W, num_elems=2 * MCOLS, num_idxs=NI2)
                mt2_sb[g2] = mt2

            emit_lg(0)
            emit_lg(1)
            for p in (0, 1):
                emit_dyn(p)
            emit_lg(2)
            for p in (0, 1):
                emit_d16(p)
            emit_scatter(0)
            for p in (2, 3):
                emit_dyn(p)
            emit_lg(3)
            for p in (2, 3):
                emit_d16(p)
            emit_scatter(1)
            for p in (4, 5, 6, 7):
                emit_dyn(p)
            emit_lg(4)
            for p in (4, 5):
                emit_d16(p)
            emit_scatter(2)
            for p in (6, 7):
                emit_d16(p)
            emit_scatter(3)
            for p in (8, 9, 10, 11, 12, 13):
                emit_dyn(p)
            for p in (8, 9):
                emit_d16(p)
            emit_scatter(4)
            for p in (10, 11):
                emit_d16(p)
            emit_scatter(5)
            for p in (12, 13):
                emit_d16(p)
            emit_scatter(6)

            # --------- PE: transpose M chunks + banded matmuls, pipelined ------
            m_sb = big.tile([PCHUNK, NPAIR, 3, 112], f16, tag="m")
            out_sb = big.tile([128, 3, NPIX], f16, tag="out")

            def emit_T(p):
                tp = psT.tile([PCHUNK, 3, 112], f16, tag="psT", name=f"tp{p}")
                src = mt2_sb[p // 2]
                base = (p % 2) * MCOLS
                for t3 in range(3):
                    nc.tensor.matmul(tp[:, t3, :],
                                     src[:, base + t3 * PCHUNK:base + (t3 + 1) * PCHUNK],
                                     ident_sb[:], start=True, stop=True,
                                     is_transpose=True)
                nc.vector.tensor_copy(m_sb[:, p, :, :], tp[:])

            def emit_B(p):
                po = psO.tile([128, 3, 112], f32, tag="psO", name=f"po{p}")
                for cc in range(3):
                    for trel in range(3):
                        nc.tensor.matmul(po[:, cc, :],
                                         xt_sb[:, p + trel, cc * 128:(cc + 1) * 128],
                                         m_sb[:, p, trel, :],
                                         start=(trel == 0), stop=(trel == 2))
                dst = out_sb[:, :, p * 112:(p + 1) * 112]
                if p % 2 == 0:
                    nc.scalar.copy(dst, po[:])
                else:
                    nc.vector.tensor_copy(dst, po[:])

            emit_T(0)
            emit_T(1)
            for p in range(2, NPAIR):
                emit_T(p)
                emit_B(p - 2)
            emit_B(NPAIR - 2)
            emit_B(NPAIR - 1)

            # ---------------- streamed output DMAs ----------------
            for off, wdt in OCH:
                nc.sync.dma_start(
                    out=out_d[:, off:off + wdt].rearrange("(u p) n -> p u n", p=128),
                    in_=out_sb[:, :, off:off + wdt])
    nc.finalize()
    return nc


def _pool_matrix():
    bt = np.zeros((3200, S * S), np.float16)
    r = np.arange(HW)
    bt[r, (r // 56 // 8) * 7 + (r % 56) // 8] = 1.0
    return np.ascontiguousarray(bt.reshape(25, 128, S * S).transpose(1, 0, 2)).reshape(128, -1)


def _static_inputs():
    sidx = np.full((2 * W, NI), -1, np.int16)
    for hl in range(2):
        for w in range(W):
            for di in range(KK):
                for dj in range(KK):
                    sidx[hl * W + w, 5 * di + dj] = (hl + di) * PADW + w + dj
    sidx2 = np.full((2 * W, NI2), -1, np.int16)
    sidx2[:, 0:NI] = sidx
    sidx2[:, NI:NI2] = np.where(sidx >= 0, sidx + MCOLS, -1).astype(np.int16)
    return sidx2


def _prep(x, ctx, Wq, Wk, Wwd):
    sidx2 = _static_inputs()
    # wkq[c', c] = (Wk^T @ Wq / 64); lhsT chunks packed [128, u, c]
    wkq = (Wk.T.astype(np.float64) @ Wq.astype(np.float64) / 64.0).astype(np.float16)
    wkq = np.ascontiguousarray(wkq.reshape(3, 128, C).transpose(1, 0, 2)).reshape(128, 3 * C)
    wwdt1 = np.concatenate([Wwd.T, np.ones((S * S, 1), np.float32)], axis=1).astype(np.float16)
    ident = np.eye(2 * W, dtype=np.float16)
    bt = _pool_matrix()
    in_maps = []
    for core in range(NCORES):
        b, half = core // 2, core % 2
        r0 = half * ROWS
        ct = np.zeros((3200, C), np.float16)
        ct[0:HW] = ctx[b].reshape(C, HW).T
        ct = np.ascontiguousarray(ct.reshape(25, 128, C).transpose(1, 0, 2)).reshape(128, -1)
        xn = np.ascontiguousarray(x[b, :, r0:r0 + ROWS, :].reshape(C, NPIX)).astype(np.float16)
        xp = np.zeros((PADR, PADW, C), np.float32)
        lo, hi = max(0, r0 - 2), min(H, r0 + ROWS + 2)
        xp[lo - (r0 - 2):hi - (r0 - 2), 2:2 + W, :] = np.transpose(x[b, :, lo:hi, :], (1, 2, 0))
        xt = xp.reshape(WSPACE, C).astype(np.float16)
        in_maps.append(dict(ct=ct, bt=bt, wkq=wkq, wwdt1=wwdt1, sidx2=sidx2,
                            ident=ident, xn=xn, xt=xt))
    return in_maps


def kernel(x, ctx, Wq, Wk, Wwd, _trace=False):
    from concourse.bass_utils import run_bass_kernel_spmd

    x, ctx = np.asarray(x), np.asarray(ctx)
    Wq, Wk, Wwd = np.asarray(Wq), np.asarray(Wk), np.asarray(Wwd)
    if "nc" not in _cached:
        _cached["nc"] = _build_nc()
    in_maps = _prep(x, ctx, Wq, Wk, Wwd)
    res = run_bass_kernel_spmd(_cached["nc"], in_maps, list(range(NCORES)), trace=_trace)
    _cached["last_result"] = res
    out = np.empty((B, C, H, W), np.float32)
    for core in range(NCORES):
        b, half = core // 2, core % 2
        r0 = half * ROWS
        out[b, :, r0:r0 + ROWS, :] = res.results[core]["out"].astype(np.float32).reshape(C, ROWS, W)
    return out


# revision 28
# speedup vs baseline: 1.1961x; 1.0939x over previous
"""ContMix kernel for TRN2, 8 NeuronCores.

Sharding: (batch b, H-half) -> 8 cores. Each core computes out[b, :, r0:r0+28, :].

Math (per batch b):
  ctx_p = avgpool8x8(ctx)                      [C, 49]   (DVE f16 tree-adds, 4x mode)
  G     = (Wq^T Wk / 64) @ ctx_p               [C, 49]   (weights folded on host)
  lg    = G^T @ x                              [49, HW]  (5 pair-aligned psum chunks)
  A     = softmax_s(lg)  (no max-sub; |lg| ~ 8), expa f16
  dynT  = A^T @ Wwd^T                          [112, 26] per row-pair
  out[c, n] = sum_j x_patch[c, j, n] * dyn[j, n]
Final step: banded matmuls on TensorE. Per row-pair a [120 x 112] banded
matrix M is built fully on-chip: gpsimd local_scatter writes M^T for TWO
pairs per call, then TensorE transpose-mode matmuls (identity rhs)
produce M chunks in PSUM (f16), copied to SBUF by DVE. Output copies on
ACT, streamed f16 output DMA, host upcasts to f32.
"""

import numpy as np

B, C, H, W = 4, 384, 56, 56
KK, S = 5, 7
NCORES = 8
ROWS = H // 2              # 28 rows per core
NPIX = ROWS * W            # 1568
PADR = ROWS + 4            # 32 padded rows
PADW = 60                  # padded width (56 + 4 halo)
WSPACE = PADR * PADW       # 1920 padded pixels
PCHUNK = 2 * PADW          # 120 partitions per contraction chunk
NPAIR = ROWS // 2          # 14 output row-pairs
NCHUNK = PADR // 2         # 16 contraction chunks
D2 = C // 2                # 192
NI = 26                    # scatter idxs per pair (25 taps + 1 pad)
NI2 = 2 * NI               # two pairs per scatter call
MCOLS = 3 * PCHUNK         # 360 = w''-space per pair
HW = H * W

_cached = {}


def _build_nc():
    import concourse.tile as tile
    from concourse import bacc, mybir, library_config

    f32, f16, i16 = mybir.dt.float32, mybir.dt.float16, mybir.dt.int16
    nc = bacc.Bacc("TRN2", target_bir_lowering=False, debug=False)

    ct_d = nc.dram_tensor("ct", [128, 25 * C], f16, kind="ExternalInput")
    bt_d = nc.dram_tensor("bt", [128, 25 * S * S], f16, kind="ExternalInput")
    wkq_d = nc.dram_tensor("wkq", [128, 3 * C], f16, kind="ExternalInput")
    wwdt1_d = nc.dram_tensor("wwdt1", [S * S, NI], f16, kind="ExternalInput")
    sidx2_d = nc.dram_tensor("sidx2", [2 * W, NI2], i16, kind="ExternalInput")
    ident_d = nc.dram_tensor("ident", [2 * W, 2 * W], f16, kind="ExternalInput")
    xn_d = nc.dram_tensor("xn", [C, NPIX], f16, kind="ExternalInput")
    xt_d = nc.dram_tensor("xt", [WSPACE, C], f16, kind="ExternalInput")
    out_d = nc.dram_tensor("out", [C, NPIX], f16, kind="ExternalOutput")

    # logits chunks (pair-aligned; small first to launch the scatter chain asap)
    LCH = [(0, 224), (224, 224), (448, 448), (896, 448), (1344, 224)]
    # output DMA chunks
    OCH = [(0, 784), (784, 560), (1344, 112), (1456, 112)]

    with tile.TileContext(nc) as tc:
        with (
            tc.tile_pool(name="big", bufs=1) as big,
            tc.tile_pool(name="wrk", bufs=4) as wrk,
            tc.tile_pool(name="mtp", bufs=3) as mtp,
            tc.tile_pool(name="psA", bufs=2, space="PSUM") as psA,
            tc.tile_pool(name="psD", bufs=2, space="PSUM") as psD,
            tc.tile_pool(name="psT", bufs=2, space="PSUM") as psT,
            tc.tile_pool(name="psO", bufs=2, space="PSUM") as psO,
        ):
            # ---------------- input DMAs (pooling chain first) ----------------
            bt_sb = big.tile([128, 25, S * S], f16, tag="bt")
            nc.sync.dma_start(out=bt_sb[:], in_=bt_d[:].rearrange("p (k s) -> p k s", k=25))
            ident_sb = big.tile([2 * W, 2 * W], f16, tag="ident")
            nc.sync.dma_start(out=ident_sb[:], in_=ident_d[:])
            ct_sb = big.tile([128, 25, C], f16, tag="ct")
            for k0, nk in ((0, 8), (8, 7), (15, 5), (20, 5)):
                nc.sync.dma_start(out=ct_sb[:, k0:k0 + nk, :],
                                  in_=ct_d[:, k0 * C:(k0 + nk) * C].rearrange(
                                      "p (k c) -> p k c", k=nk))
            wkq_sb = big.tile([128, 3, C], f16, tag="wkq")
            nc.sync.dma_start(out=wkq_sb[:], in_=wkq_d[:].rearrange("p (u c) -> p u c", u=3))
            wwdt1_sb = big.tile([S * S, NI], f16, tag="wwdt1")
            nc.sync.dma_start(out=wwdt1_sb[:], in_=wwdt1_d[:])
            sidx2_sb = big.tile([2 * W, NI2], i16, tag="sidx2")
            nc.sync.dma_start(out=sidx2_sb[:], in_=sidx2_d[:])
            xn_sb = big.tile([128, 3, NPIX], f16, tag="xn")
            for off, wdt in ((0, 448), (448, 448), (896, 448), (1344, 224)):
                nc.sync.dma_start(out=xn_sb[:, :, off:off + wdt],
                                  in_=xn_d[:, off:off + wdt].rearrange("(u p) n -> p u n", p=128))
            xt_sb = big.tile([PCHUNK, NCHUNK, C], f16, tag="xt")
            nc.sync.dma_start(out=xt_sb[:, 0:8, :],
                              in_=xt_d[0:8 * PCHUNK, :].rearrange("(t p) c -> p t c", p=PCHUNK))
            nc.sync.dma_start(out=xt_sb[:, 8:16, :],
                              in_=xt_d[8 * PCHUNK:, :].rearrange("(t p) c -> p t c", p=PCHUNK))

            nc.gpsimd.load_library(library_config.local_scatter)

            # -------- pooling on TensorE: ctx_p^T = B^T @ ctx^T ---------------
            # (contraction over hw positions; one-hot bin matrix B, 25 chunks)
            pool_ps = psA.tile([S * S, C], f32, tag="psA", name="pool_ps")
            for k in range(25):
                nc.tensor.matmul(pool_ps[:], bt_sb[:, k, :], ct_sb[:, k, :],
                                 start=(k == 0), stop=(k == 24))
            ctxpT = big.tile([S * S, C], f16, tag="ctxpT")
            nc.vector.tensor_copy(ctxpT[:], pool_ps[:])
            # transpose [49, 384] -> [384, 49] via 3 PE transpose matmuls
            ctp_ps = psT.tile([128, 3, 50], f16, tag="psT", name="ctp_ps")
            for cb in range(3):
                nc.tensor.matmul(ctp_ps[:, cb, 0:S * S], ctxpT[:, cb * 128:(cb + 1) * 128],
                                 ident_sb[0:S * S, 0:S * S], start=True, stop=True,
                                 is_transpose=True)
            ctx_p = big.tile([128, 3, S * S], f16, tag="ctxp")
            nc.vector.tensor_copy(ctx_p[:], ctp_ps[:, :, 0:S * S])

            # ------------- G = (Wq^T Wk/64) @ ctx_p : [384, 49], f16 ----------
            g_sb = big.tile([128, 3, S * S], f16, tag="g")
            g_ps = psA.tile([128, 3, S * S], f32, tag="psA", name="g_ps")
            for cb in range(3):
                for u in range(3):
                    nc.tensor.matmul(g_ps[:, cb, :], wkq_sb[:, u, cb * 128:(cb + 1) * 128],
                                     ctx_p[:, u, :], start=(u == 0), stop=(u == 2))
            nc.scalar.copy(g_sb[:], g_ps[:])

            # ------------- logits + exp (f16), dyn/d16 interleaved ------------
            expa = big.tile([S * S, NPIX], f16, tag="expa")
            dyn_ps = [None] * NPAIR
            d2_sb = [None] * (NPAIR // 2)

            def emit_lg(k):
                off, wdt = LCH[k]
                lg = psA.tile([S * S, 448], f32, tag="psA", name=f"lg{k}")
                for u in range(3):
                    nc.tensor.matmul(lg[:, 0:wdt], g_sb[:, u, :], xn_sb[:, u, off:off + wdt],
                                     start=(u == 0), stop=(u == 2))
                nc.scalar.activation(expa[:, off:off + wdt], lg[:, 0:wdt],
                                     mybir.ActivationFunctionType.Exp)

            def emit_dyn(p):
                dp = psD.tile([2 * W, NI], f32, tag="psD", name=f"dyn{p}")
                nc.tensor.matmul(dp[:], expa[:, p * 112:(p + 1) * 112],
                                 wwdt1_sb[:], start=True, stop=True)
                dyn_ps[p] = dp

            def emit_d16(p):
                g2 = p // 2
                if d2_sb[g2] is None:
                    d2_sb[g2] = wrk.tile([2 * W, NI2], f16, tag="d2", name=f"d2_{g2}")
                rec = wrk.tile([2 * W, 1], f32, tag="rec", name=f"rec{p}")
                nc.vector.reciprocal(rec[:], dyn_ps[p][:, 25:26])
                nc.vector.tensor_scalar_mul(
                    d2_sb[g2][:, (p % 2) * NI:(p % 2 + 1) * NI], dyn_ps[p][:], rec[:])

            mt2_sb = [None] * (NPAIR // 2)

            def emit_scatter(g2):
                mt2 = mtp.tile([2 * W, 2 * MCOLS], f16, tag="mt2", name=f"mt2_{g2}")
                nc.gpsimd.local_scatter(mt2[:], d2_sb[g2][:], sidx2_sb[:],
                                        channels=2 * W, num_elems=2 * MCOLS, num_idxs=NI2)
                mt2_sb[g2] = mt2

            emit_lg(0)
            emit_lg(1)
            for p in (0, 1):
                emit_dyn(p)
            emit_lg(2)
            for p in (0, 1):
                emit_d16(p)
            emit_scatter(0)
            for p in (2, 3):
                emit_dyn(p)
            emit_lg(3)
            for p in (2, 3):
                emit_d16(p)
            emit_scatter(1)
            for p in (4, 5, 6, 7):
                emit_dyn(p)
            emit_lg(4)
            for p in (4, 5):
                emit_d16(p)
            emit_scatter(2)
            for p in (6, 7):
                emit_d16(p)
            emit_scatter(3)
            for p in (8, 9, 10, 11, 12, 13):
                emit_dyn(p)
            for p in (8, 9):
                emit_d16(p)
            emit_scatter(4)
            for p in (10, 11):
                emit_d16(p)
            emit_scatter(5)
            for p in (12, 13):
                emit_d16(p)
            emit_scatter(6)

            # --------- PE: transpose M chunks + banded matmuls, pipelined ------
            m_sb = big.tile([PCHUNK, NPAIR, 3, 112], f16, tag="m")
            out_sb = big.tile([128, 3, NPIX], f16, tag="out")

            def emit_T(p):
                tp = psT.tile([PCHUNK, 3, 112], f16, tag="psT", name=f"tp{p}")
                src = mt2_sb[p // 2]
                base = (p % 2) * MCOLS
                for t3 in range(3):
                    nc.tensor.matmul(tp[:, t3, :],
                                     src[:, base + t3 * PCHUNK:base + (t3 + 1) * PCHUNK],
                                     ident_sb[:], start=True, stop=True,
                                     is_transpose=True)
                nc.vector.tensor_copy(m_sb[:, p, :, :], tp[:])

            def emit_B(p):
                po = psO.tile([128, 3, 112], f32, tag="psO", name=f"po{p}")
                for cc in range(3):
                    for trel in range(3):
                        nc.tensor.matmul(po[:, cc, :],
                                         xt_sb[:, p + trel, cc * 128:(cc + 1) * 128],
                                         m_sb[:, p, trel, :],
                                         start=(trel == 0), stop=(trel == 2))
                nc.scalar.copy(out_sb[:, :, p * 112:(p + 1) * 112], po[:])

            emit_T(0)
            emit_T(1)
            for p in range(2, NPAIR):
                emit_T(p)
                emit_B(p - 2)
            emit_B(NPAIR - 2)
            emit_B(NPAIR - 1)

            # ---------------- streamed output DMAs ----------------
            for off, wdt in OCH:
                nc.sync.dma_start(
                    out=out_d[:, off:off + wdt].rearrange("(u p) n -> p u n", p=128),
                    in_=out_sb[:, :, off:off + wdt])
    nc.finalize()
    return nc


def _pool_matrix():
    bt = np.zeros((3200, S * S), np.float16)
    r = np.arange(HW)
    bt[r, (r // 56 // 8) * 7 + (r % 56) // 8] = 1.0
    return np.ascontiguousarray(bt.reshape(25, 128, S * S).transpose(1, 0, 2)).reshape(128, -1)


def _static_inputs():
    sidx = np.full((2 * W, NI), -1, np.int16)
    for hl in range(2):
        for w in range(W):
            for di in range(KK):
                for dj in range(KK):
                    sidx[hl * W + w, 5 * di + dj] = (hl + di) * PADW + w + dj
    sidx2 = np.full((2 * W, NI2), -1, np.int16)
    sidx2[:, 0:NI] = sidx
    sidx2[:, NI:NI2] = np.where(sidx >= 0, sidx + MCOLS, -1).astype(np.int16)
    return sidx2


def _prep(x, ctx, Wq, Wk, Wwd):
    sidx2 = _static_inputs()
    # wkq[c', c] = (Wk^T @ Wq / 64); lhsT chunks packed [128, u, c]
    wkq = (Wk.T.astype(np.float64) @ Wq.astype(np.float64) / 64.0).astype(np.float16)
    wkq = np.ascontiguousarray(wkq.reshape(3, 128, C).transpose(1, 0, 2)).reshape(128, 3 * C)
    wwdt1 = np.concatenate([Wwd.T, np.ones((S * S, 1), np.float32)], axis=1).astype(np.float16)
    ident = np.eye(2 * W, dtype=np.float16)
    bt = _pool_matrix()
    in_maps = []
    for core in range(NCORES):
        b, half = core // 2, core % 2
        r0 = half * ROWS
        ct = np.zeros((3200, C), np.float16)
        ct[0:HW] = ctx[b].reshape(C, HW).T
        ct = np.ascontiguousarray(ct.reshape(25, 128, C).transpose(1, 0, 2)).reshape(128, -1)
        xn = np.ascontiguousarray(x[b, :, r0:r0 + ROWS, :].reshape(C, NPIX)).astype(np.float16)
        xp = np.zeros((PADR, PADW, C), np.float32)
        lo, hi = max(0, r0 - 2), min(H, r0 + ROWS + 2)
        xp[lo - (r0 - 2):hi - (r0 - 2), 2:2 + W, :] = np.transpose(x[b, :, lo:hi, :], (1, 2, 0))
        xt = xp.reshape(WSPACE, C).astype(np.float16)
        in_maps.append(dict(ct=ct, bt=bt, wkq=wkq, wwdt1=wwdt1, sidx2=sidx2,
                            ident=ident, xn=xn, xt=xt))
    return in_maps


def kernel(x, ctx, Wq, Wk, Wwd, _trace=False):
    from concourse.bass_utils import run_bass_kernel_spmd

    x, ctx = np.asarray(x), np.asarray(ctx)
    Wq, Wk, Wwd = np.asarray(Wq), np.asarray(Wk), np.asarray(Wwd)
    if "nc" not in _cached:
        _cached["nc"] = _build_nc()
    in_maps = _prep(x, ctx, Wq, Wk, Wwd)
    res = run_bass_kernel_spmd(_cached["nc"], in_maps, list(range(NCORES)), trace=_trace)
    _cached["last_result"] = res
    out = np.empty((B, C, H, W), np.float32)
    for core in range(NCORES):
        b, half = core // 2, core % 2
        r0 = half * ROWS
        out[b, :, r0:r0 + ROWS, :] = res.results[core]["out"].astype(np.float32).reshape(C, ROWS, W)
    return out
